# revision 1
# baseline (speedup 1.0000x reference)
"""CBIndirectionLookup Trainium2 kernel.

Problem: x [N=2097152, 8] int32 bit-vectors; patterns [256, 8] (unique bit rows);
results [256, 4] int32. Output: results[argmax(all(x==patterns))] -> [N, 4] int32.

Strategy (pure data-parallel over 8 cores, N/8 = 262144 elems each):
 - Host: fold patterns+results into two 128-entry fp16 lut halves addressed by a
   7-bit code c7 (Horner over bits 0..6) plus sign sigma = 1-2*b7:
       out = Q[c7] + sigma * P[c7],  P = (lo-hi)/2, Q = (lo+hi)/2
 - Device per core, blocks of 16384 elements (x tile [128, 1024] i32):
   1. DVE tensor_tensor_scan (Horner) -> codes c7 (slot 6 of 8) and b7 (slot 7).
   2. PE transpose of strided c7 view -> code rows; GPSIMD partition_broadcast
      materializes each code row across 128 partitions (fp16).
   3. DVE tensor_scalar is_equal(bcast_codes, iota) -> one-hot weights W (fp16).
   4. PE: ldweights W[128,128] + matmul rhs=[P|Q] [128,8] -> psum [128 elems, 8].
   5. DVE recombine: out = Q + sigma*P (sigma natural layout) -> int32 sbuf.
   6. DMA out.
"""
import sys
sys.path.insert(0, "/opt/trn_rl_repo")

import numpy as np

N = 2_097_152
W_IN = 8
W_OUT = 4
P_TAB = 256
N_CORES = 8
N_LOC = N // N_CORES            # 262144 elements per core
BLK_ELEMS = 16384               # elements per block: [128, 128] layout
N_BLKS = N_LOC // BLK_ELEMS     # 16
EPP = BLK_ELEMS // 128          # 128 elements per partition per block
GROUP_BLKS = 4                  # blocks per code-transpose group (psum bank = 512)
N_GROUPS = N_BLKS // GROUP_BLKS


def _build_luts(patterns: np.ndarray, results: np.ndarray):
    """Host-side: fold the tiny tables into P/Q fp16 lut halves keyed by c7."""
    pat2idx = {}
    for p in range(patterns.shape[0]):
        pat2idx[tuple(int(v) for v in patterns[p])] = p
    lo = np.zeros((128, W_OUT), np.float64)
    hi = np.zeros((128, W_OUT), np.float64)
    for q in range(128):
        bits = [(q >> j) & 1 for j in range(7)]  # c7 = sum_j b_j 2^j
        p0 = pat2idx[tuple(bits + [0])]
        p1 = pat2idx[tuple(bits + [1])]
        lo[q] = results[p0]
        hi[q] = results[p1]
    Pm = (lo - hi) / 2.0
    Qm = (lo + hi) / 2.0
    rhs = np.concatenate([Pm, Qm], axis=1).astype(np.float16)  # [128, 8]
    assert np.array_equal(rhs.astype(np.float64), np.concatenate([Pm, Qm], 1))
    return rhs


def _build_kernel(repeats=1):
    import concourse.bass as bass
    import concourse.bacc as bacc
    import concourse.tile as tile
    from concourse import mybir

    nc = bacc.Bacc("TRN2", target_bir_lowering=False, debug=False,
                   num_devices=N_CORES)
    dt = mybir.dt
    x = nc.dram_tensor("x", [128, N_LOC // 128 * W_IN], dt.int32,
                       kind="ExternalInput").ap()
    luts = nc.dram_tensor("luts", [128, 8], dt.float16,
                          kind="ExternalInput").ap()
    y = nc.dram_tensor("y", [128, N_LOC // 128 * W_OUT], dt.int32,
                       kind="ExternalOutput").ap()
    ct_scratch = nc.dram_tensor("ct_scratch", [2, 128, GROUP_BLKS * 128],
                                dt.float16).ap()
    # x layout: core-row-major; partition p of block b holds elements
    #   n_loc = b*16384 + p*128 + i  (i in [0,128))
    # x dram row p = [block0: 128*8 | block1: 128*8 | ...] int32
    # y dram row p likewise with stride 4.

    with tile.TileContext(nc) as tc:
        with (
            tc.tile_pool(name="const", bufs=1) as constp,
            tc.tile_pool(name="xin", bufs=3) as xinp,
            tc.tile_pool(name="scan", bufs=3) as scanp,
            tc.tile_pool(name="sig", bufs=8) as sigp,
            tc.tile_pool(name="ct", bufs=2) as ctp,
            tc.tile_pool(name="ctr", bufs=2) as ctrp,
            tc.tile_pool(name="bc", bufs=6) as bcp,
            tc.tile_pool(name="w", bufs=6) as wp,
            tc.tile_pool(name="out", bufs=3) as outp,
            tc.tile_pool(name="pct", bufs=1, space="PSUM") as pctp,
            tc.tile_pool(name="pobc", bufs=3, space="PSUM") as pobc,
            tc.tile_pool(name="pg", bufs=1, space="PSUM") as pgp,
        ):
            t_luts = constp.tile([128, 8], dt.float16)
            nc.sync.dma_start(out=t_luts[:], in_=luts[:])
            t_iota32 = constp.tile([128, 1], dt.int32)
            nc.gpsimd.iota(t_iota32[:], pattern=[[0, 1]], base=0,
                           channel_multiplier=1)
            t_iota = constp.tile([128, 1], dt.float32)
            nc.vector.tensor_copy(t_iota[:], t_iota32[:])
            t_ident = constp.tile([128, 128], dt.float32)
            from concourse.masks import make_identity
            make_identity(nc, t_ident[:])
            # bit-weight pattern [1,2,4,...,64,0] per 8-group (c7 = sum b_j 2^j)
            t_pat = constp.tile([128, EPP * 8], dt.int32)
            for j in range(8):
                nc.vector.memset(
                    t_pat[:].rearrange("p (e k) -> p e k", k=8)[:, :, j:j + 1],
                    float(2 ** j) if j < 7 else 0.0)
            t_ones = constp.tile([1, 128], dt.float16)
            nc.vector.memset(t_ones[:], 1.0)

            import contextlib
            rep_cm = (tc.For_i(0, repeats, 1) if repeats > 1
                      else contextlib.nullcontext())
            with rep_cm:
              for g in range(N_GROUPS):
                  # --- per group: 4 blocks -> wide transposed code rows ---
                  t_ct_ps = pctp.tile([128, GROUP_BLKS * 128], dt.float32)
                  sigs = []
                  for bb in range(GROUP_BLKS):
                      b = g * GROUP_BLKS + bb
                      t_x = xinp.tile([128, EPP * 8], dt.int32)
                      nc.sync.dma_start(
                          out=t_x[:],
                          in_=x[:, b * EPP * 8:(b + 1) * EPP * 8])
                      x3 = t_x[:].rearrange("p (e k) -> p e k", k=8)
                      t_xw = scanp.tile([128, EPP * 8], dt.float32)
                      nc.vector.tensor_tensor(
                          out=t_xw[:], in0=t_x[:], in1=t_pat[:],
                          op=mybir.AluOpType.mult)
                      t_c7 = scanp.tile([128, EPP], dt.float32, tag="c7")
                      nc.vector.tensor_reduce(
                          t_c7[:], t_xw[:].rearrange("p (e k) -> p e k", k=8),
                          axis=mybir.AxisListType.X, op=mybir.AluOpType.add)
                      # sigma = 1 - 2*b7 (fp32) [128, EPP]
                      t_sg = sigp.tile([128, EPP], dt.float32)
                      nc.vector.tensor_scalar(
                          out=t_sg[:], in0=x3[:, :, 7],
                          scalar1=-2.0, scalar2=1.0,
                          op0=mybir.AluOpType.mult, op1=mybir.AluOpType.add)
                      sigs.append(t_sg)
                      # transpose c7 into group psum columns
                      nc.tensor.transpose(
                          t_ct_ps[:, bb * 128:(bb + 1) * 128],
                          t_c7[:], t_ident[:])
                  # escape transposed codes to sbuf fp16 rows
                  t_ct = ctp.tile([128, GROUP_BLKS * 128], dt.float16)
                  nc.vector.tensor_copy(t_ct[:], t_ct_ps[:])
                  nc.sync.dma_start(out=ct_scratch[g % 2], in_=t_ct[:])

                  # --- per code-row i: bcast, one-hot, 4 gather matmuls ---
                  t_pg = []
                  for par in range(2):
                      t_pg_buf = pgp.tile([128, 1024], dt.float32, tag=f"pg{par}")
                      t_pg.append(t_pg_buf)
                  ROWS_PER_Q = 32
                  for i in range(128):
                      if i % ROWS_PER_Q == 0:
                          t_ctrow = ctrp.tile([1, ROWS_PER_Q * GROUP_BLKS * 128],
                                              dt.float16, tag="ctrow")
                          nc.sync.dma_start(
                              out=t_ctrow[:],
                              in_=ct_scratch[g % 2, i:i + ROWS_PER_Q, :].rearrange(
                                  "r f -> (r f)")[None, :])
                      W_ROW = GROUP_BLKS * 128
                      row = t_ctrow[0:1,
                                    (i % ROWS_PER_Q) * W_ROW:(i % ROWS_PER_Q + 1) * W_ROW]
                      t_w = wp.tile([128, GROUP_BLKS * 128], dt.float16)
                      if i % 16 < 10:
                          t_bc = bcp.tile([128, GROUP_BLKS * 128], dt.float16)
                          nc.gpsimd.partition_broadcast(t_bc[:], row)
                          nc.vector.tensor_scalar(
                              out=t_w[:], in0=t_bc[:], scalar1=t_iota[:],
                              scalar2=None, op0=mybir.AluOpType.is_equal)
                      else:
                          t_obc = pobc.tile([128, GROUP_BLKS * 128], dt.float32)
                          nc.tensor.matmul(t_obc[:], t_ones[:], row,
                                           start=True, stop=True)
                          nc.vector.tensor_scalar(
                              out=t_w[:], in0=t_obc[:], scalar1=t_iota[:],
                              scalar2=None, op0=mybir.AluOpType.is_equal)
                      for bb in range(GROUP_BLKS):
                          ps = t_pg[bb % 2]
                          half = (bb // 2) * 512
                          nc.tensor.matmul(
                              ps[:, half + (i % 64) * 8: half + (i % 64) * 8 + 8],
                              t_w[:, bb * 128:(bb + 1) * 128],
                              t_luts[:],
                              start=True, stop=True)
                      # drain psum every 64 rows: recombine + dma out
                      if i % 64 == 63:
                          i0 = i - 63
                          for bb in range(GROUP_BLKS):
                              b = g * GROUP_BLKS + bb
                              ps = t_pg[bb % 2]
                              half = (bb // 2) * 512
                              pview = ps[:, half:half + 512].rearrange(
                                  "p (i w) -> p i w", w=8)
                              t_o = outp.tile([128, 64 * 4], dt.int32,
                                              tag="o")
                              o3 = t_o[:].rearrange("p (i w) -> p i w", w=4)
                              sg = sigs[bb]
                              sg4 = bass.AP(
                                  tensor=sg.tensor, offset=sg[:, i0:i0 + 64].offset,
                                  ap=sg[:, i0:i0 + 64].ap + [[0, 4]])
                              t_t = outp.tile([128, 64 * 4], dt.float32,
                                              tag="t")
                              t3 = t_t[:].rearrange("p (i w) -> p i w", w=4)
                              # t = sigma * P ; out = t + Q
                              nc.vector.tensor_tensor(
                                  out=t3[:, :, :], in0=pview[:, :, 0:4],
                                  in1=sg4, op=mybir.AluOpType.mult)
                              nc.vector.tensor_tensor(
                                  out=o3[:, :, :], in0=t3[:, :, :],
                                  in1=pview[:, :, 4:8],
                                  op=mybir.AluOpType.add)
                              # dma out: partition p, elements b*16384+p*128+(i0..i0+63)
                              nc.sync.dma_start(
                                  out=y[:, (b * EPP + i0) * 4:(b * EPP + i0 + 64) * 4],
                                  in_=t_o[:])
    nc.compile()
    return nc


_CACHE = {}


def kernel(x: np.ndarray, patterns: np.ndarray, results: np.ndarray) -> np.ndarray:
    import jax
    from jax.sharding import Mesh, PartitionSpec, NamedSharding
    from jax.experimental.shard_map import shard_map
    from concourse import mybir
    from concourse.bass2jax import (_bass_exec_p, install_neuronx_cc_hook,
                                    partition_id_tensor)

    x = np.asarray(x)
    patterns = np.asarray(patterns)
    results = np.asarray(results)
    rhs_luts = _build_luts(patterns, results)

    if "nc" not in _CACHE:
        _CACHE["nc"] = _build_kernel()
    nc = _CACHE["nc"]

    install_neuronx_cc_hook()
    partition_name = nc.partition_id_tensor.name if nc.partition_id_tensor else None
    in_names, out_names, out_avals, zero_outs = [], [], [], []
    for alloc in nc.m.functions[0].allocations:
        if not isinstance(alloc, mybir.MemoryLocationSet):
            continue
        name = alloc.memorylocations[0].name
        if alloc.kind == "ExternalInput":
            if name != partition_name:
                in_names.append(name)
        elif alloc.kind == "ExternalOutput":
            out_names.append(name)
            shape = tuple(alloc.tensor_shape)
            dtype = mybir.dt.np(alloc.dtype)
            out_avals.append(jax.core.ShapedArray(shape, dtype))
            zero_outs.append(np.zeros(shape, dtype))
    n_params = len(in_names)
    n_outs = len(out_avals)
    all_in_names = in_names + out_names + ([partition_name] if partition_name else [])

    def _body(*args):
        operands = list(args)
        if partition_name is not None:
            operands.append(partition_id_tensor())
        outs = _bass_exec_p.bind(
            *operands, out_avals=tuple(out_avals), in_names=tuple(all_in_names),
            out_names=tuple(out_names), lowering_input_output_aliases=(),
            sim_require_finite=False, sim_require_nnan=False, nc=nc)
        return tuple(outs)

    devices = jax.devices()[:N_CORES]
    mesh = Mesh(np.asarray(devices), ("core",))
    shard = NamedSharding(mesh, PartitionSpec("core"))
    fn = jax.jit(
        shard_map(_body, mesh=mesh,
                  in_specs=(PartitionSpec("core"),) * (n_params + n_outs),
                  out_specs=(PartitionSpec("core"),) * n_outs,
                  check_rep=False),
        keep_unused=True)

    # Build per-core input planes. Element n_loc = b*16384 + p*128 + i maps to
    # global n = core*N_LOC + n_loc; x dram row p = concat over b of 128 rows.
    xc = x.reshape(N_CORES, N_BLKS, 128, EPP * W_IN)          # [c, b, p, 128*8]
    x_in = np.ascontiguousarray(xc.transpose(0, 2, 1, 3)).reshape(
        N_CORES * 128, N_BLKS * EPP * W_IN)
    luts_in = np.broadcast_to(rhs_luts, (N_CORES, 128, 8)).reshape(
        N_CORES * 128, 8)
    arrays = {"x": x_in.astype(np.int32), "luts": np.ascontiguousarray(luts_in)}
    args = [jax.device_put(arrays[nm], shard) for nm in in_names]
    args += [jax.device_put(
        np.zeros((N_CORES * z.shape[0], *z.shape[1:]), z.dtype), shard)
        for z in zero_outs]
    out_arrs = fn(*args)
    yi = out_names.index("y")
    yv = np.asarray(out_arrs[yi]).reshape(N_CORES, 128, N_BLKS, EPP * W_OUT)
    # invert layout: [c, p, b, 128*4] -> [c, b, p, i, 4] -> n
    y_full = yv.transpose(0, 2, 1, 3).reshape(N, W_OUT)
    return y_full.astype(np.int32)



# revision 13
# speedup vs baseline: 10032.3983x; 10032.3983x over previous
"""CBIndirectionLookup Trainium2 kernel.

Problem: x [N=2097152, 8] int32 bit-vectors; patterns [256, 8] (unique bit rows);
results [256, 4] int32. Output: results[argmax(all(x==patterns))] -> [N, 4] int32.

Strategy (pure data-parallel over 8 cores, N/8 = 262144 elems each):
 - Host: fold patterns+results into two 128-entry fp16 lut halves addressed by a
   7-bit code c7 (Horner over bits 0..6) plus sign sigma = 1-2*b7:
       out = Q[c7] + sigma * P[c7],  P = (lo-hi)/2, Q = (lo+hi)/2
 - Device per core, blocks of 16384 elements (x tile [128, 1024] i32):
   1. DVE tensor_tensor_scan (Horner) -> codes c7 (slot 6 of 8) and b7 (slot 7).
   2. PE transpose of strided c7 view -> code rows; GPSIMD partition_broadcast
      materializes each code row across 128 partitions (fp16).
   3. DVE tensor_scalar is_equal(bcast_codes, iota) -> one-hot weights W (fp16).
   4. PE: ldweights W[128,128] + matmul rhs=[P|Q] [128,8] -> psum [128 elems, 8].
   5. DVE recombine: out = Q + sigma*P (sigma natural layout) -> int32 sbuf.
   6. DMA out.
"""
import sys
sys.path.insert(0, "/opt/trn_rl_repo")

import numpy as np

N = 2_097_152
W_IN = 8
W_OUT = 4
P_TAB = 256
N_CORES = 8
N_LOC = N // N_CORES            # 262144 elements per core
BLK_ELEMS = 16384               # elements per block: [128, 128] layout
N_BLKS = N_LOC // BLK_ELEMS     # 16
EPP = BLK_ELEMS // 128          # 128 elements per partition per block
GROUP_BLKS = 4                  # blocks per code-transpose group (psum bank = 512)
N_GROUPS = N_BLKS // GROUP_BLKS


def _build_luts(patterns: np.ndarray, results: np.ndarray):
    """Host-side: fold the tiny tables into P/Q fp16 lut halves keyed by c7."""
    pat2idx = {}
    for p in range(patterns.shape[0]):
        pat2idx[tuple(int(v) for v in patterns[p])] = p
    lo = np.zeros((128, W_OUT), np.float64)
    hi = np.zeros((128, W_OUT), np.float64)
    for q in range(128):
        bits = [(q >> j) & 1 for j in range(7)]  # c7 = sum_j b_j 2^j
        p0 = pat2idx[tuple(bits + [0])]
        p1 = pat2idx[tuple(bits + [1])]
        lo[q] = results[p0]
        hi[q] = results[p1]
    Pm = (lo - hi) / 2.0
    Qm = (lo + hi) / 2.0
    rhs = np.concatenate([Pm, Qm], axis=1).astype(np.float16)  # [128, 8]
    assert np.array_equal(rhs.astype(np.float64), np.concatenate([Pm, Qm], 1))
    return rhs


def _build_kernel(repeats=1):
    import concourse.bass as bass
    import concourse.bacc as bacc
    import concourse.tile as tile
    from concourse import mybir

    nc = bacc.Bacc("TRN2", target_bir_lowering=False, debug=False,
                   num_devices=N_CORES)
    dt = mybir.dt
    x = nc.dram_tensor("x", [128, N_LOC // 128 * W_IN], dt.int32,
                       kind="ExternalInput").ap()
    luts = nc.dram_tensor("luts", [128, 8], dt.float16,
                          kind="ExternalInput").ap()
    y = nc.dram_tensor("y", [128, N_LOC // 128 * W_OUT], dt.int32,
                       kind="ExternalOutput").ap()
    ct_scratch = nc.dram_tensor("ct_scratch", [2, 128, GROUP_BLKS * 128],
                                dt.float16).ap()
    # x layout: core-row-major; partition p of block b holds elements
    #   n_loc = b*16384 + p*128 + i  (i in [0,128))
    # x dram row p = [block0: 128*8 | block1: 128*8 | ...] int32
    # y dram row p likewise with stride 4.

    with tile.TileContext(nc) as tc:
        with (
            tc.tile_pool(name="const", bufs=1) as constp,
            tc.tile_pool(name="xin", bufs=3) as xinp,
            tc.tile_pool(name="scan", bufs=3) as scanp,
            tc.tile_pool(name="sig", bufs=8) as sigp,
            tc.tile_pool(name="ct", bufs=2) as ctp,
            tc.tile_pool(name="ctr", bufs=2) as ctrp,
            tc.tile_pool(name="bc", bufs=6) as bcp,
            tc.tile_pool(name="w", bufs=6) as wp,
            tc.tile_pool(name="out", bufs=3) as outp,
            tc.tile_pool(name="pct", bufs=1, space="PSUM") as pctp,
            tc.tile_pool(name="pobc", bufs=3, space="PSUM") as pobc,
            tc.tile_pool(name="pg", bufs=1, space="PSUM") as pgp,
        ):
            t_luts = constp.tile([128, 8], dt.float16)
            nc.sync.dma_start(out=t_luts[:], in_=luts[:])
            t_iota32 = constp.tile([128, 1], dt.int32)
            nc.gpsimd.iota(t_iota32[:], pattern=[[0, 1]], base=0,
                           channel_multiplier=1)
            t_iota = constp.tile([128, 1], dt.float32)
            nc.vector.tensor_copy(t_iota[:], t_iota32[:])
            t_ident = constp.tile([128, 128], dt.float32)
            from concourse.masks import make_identity
            make_identity(nc, t_ident[:])
            # bit-weight pattern [1,2,4,...,64,0] per 8-group (c7 = sum b_j 2^j)
            t_pat = constp.tile([128, EPP * 8], dt.int32)
            for j in range(8):
                nc.vector.memset(
                    t_pat[:].rearrange("p (e k) -> p e k", k=8)[:, :, j:j + 1],
                    float(2 ** j) if j < 7 else 0.0)
            t_ones = constp.tile([1, 128], dt.float16)
            nc.vector.memset(t_ones[:], 1.0)

            import contextlib
            rep_cm = (tc.For_i(0, repeats, 1) if repeats > 1
                      else contextlib.nullcontext())
            with rep_cm:
              for g in range(N_GROUPS):
                  # --- per group: 4 blocks -> wide transposed code rows ---
                  t_ct_ps = pctp.tile([128, GROUP_BLKS * 128], dt.float32)
                  sigs = []
                  for bb in range(GROUP_BLKS):
                      b = g * GROUP_BLKS + bb
                      t_x = xinp.tile([128, EPP * 8], dt.int32)
                      nc.sync.dma_start(
                          out=t_x[:],
                          in_=x[:, b * EPP * 8:(b + 1) * EPP * 8])
                      x3 = t_x[:].rearrange("p (e k) -> p e k", k=8)
                      t_xw = scanp.tile([128, EPP * 8], dt.float32)
                      nc.vector.tensor_tensor(
                          out=t_xw[:], in0=t_x[:], in1=t_pat[:],
                          op=mybir.AluOpType.mult)
                      t_c7 = scanp.tile([128, EPP], dt.float32, tag="c7")
                      nc.vector.tensor_reduce(
                          t_c7[:], t_xw[:].rearrange("p (e k) -> p e k", k=8),
                          axis=mybir.AxisListType.X, op=mybir.AluOpType.add)
                      # sigma = 1 - 2*b7 (fp32) [128, EPP]
                      t_sg = sigp.tile([128, EPP], dt.float32)
                      nc.vector.tensor_scalar(
                          out=t_sg[:], in0=x3[:, :, 7],
                          scalar1=-2.0, scalar2=1.0,
                          op0=mybir.AluOpType.mult, op1=mybir.AluOpType.add)
                      sigs.append(t_sg)
                      # transpose c7 into group psum columns
                      nc.tensor.transpose(
                          t_ct_ps[:, bb * 128:(bb + 1) * 128],
                          t_c7[:], t_ident[:])
                  # escape transposed codes to sbuf fp16 rows
                  t_ct = ctp.tile([128, GROUP_BLKS * 128], dt.float16)
                  nc.vector.tensor_copy(t_ct[:], t_ct_ps[:])
                  nc.sync.dma_start(out=ct_scratch[g % 2], in_=t_ct[:])

                  # --- per code-row i: bcast, one-hot, 4 gather matmuls ---
                  t_pg = []
                  for par in range(2):
                      t_pg_buf = pgp.tile([128, 1024], dt.float32, tag=f"pg{par}")
                      t_pg.append(t_pg_buf)
                  ROWS_PER_Q = 32
                  for i in range(128):
                      if i % ROWS_PER_Q == 0:
                          t_ctrow = ctrp.tile([1, ROWS_PER_Q * GROUP_BLKS * 128],
                                              dt.float16, tag="ctrow")
                          nc.sync.dma_start(
                              out=t_ctrow[:],
                              in_=ct_scratch[g % 2, i:i + ROWS_PER_Q, :].rearrange(
                                  "r f -> (r f)")[None, :])
                      W_ROW = GROUP_BLKS * 128
                      row = t_ctrow[0:1,
                                    (i % ROWS_PER_Q) * W_ROW:(i % ROWS_PER_Q + 1) * W_ROW]
                      t_w = wp.tile([128, GROUP_BLKS * 128], dt.float16)
                      if i % 16 < 10:
                          t_bc = bcp.tile([128, GROUP_BLKS * 128], dt.float16)
                          nc.gpsimd.partition_broadcast(t_bc[:], row)
                          nc.vector.tensor_scalar(
                              out=t_w[:], in0=t_bc[:], scalar1=t_iota[:],
                              scalar2=None, op0=mybir.AluOpType.is_equal)
                      else:
                          t_obc = pobc.tile([128, GROUP_BLKS * 128], dt.float32)
                          nc.tensor.matmul(t_obc[:], t_ones[:], row,
                                           start=True, stop=True)
                          nc.vector.tensor_scalar(
                              out=t_w[:], in0=t_obc[:], scalar1=t_iota[:],
                              scalar2=None, op0=mybir.AluOpType.is_equal)
                      for bb in range(GROUP_BLKS):
                          ps = t_pg[bb % 2]
                          half = (bb // 2) * 512
                          nc.tensor.matmul(
                              ps[:, half + (i % 64) * 8: half + (i % 64) * 8 + 8],
                              t_w[:, bb * 128:(bb + 1) * 128],
                              t_luts[:],
                              start=True, stop=True)
                      # drain psum every 64 rows: recombine + dma out
                      if i % 64 == 63:
                          i0 = i - 63
                          for bb in range(GROUP_BLKS):
                              b = g * GROUP_BLKS + bb
                              ps = t_pg[bb % 2]
                              half = (bb // 2) * 512
                              pview = ps[:, half:half + 512].rearrange(
                                  "p (i w) -> p i w", w=8)
                              t_o = outp.tile([128, 64 * 4], dt.int32,
                                              tag="o")
                              o3 = t_o[:].rearrange("p (i w) -> p i w", w=4)
                              sg = sigs[bb]
                              sg4 = bass.AP(
                                  tensor=sg.tensor, offset=sg[:, i0:i0 + 64].offset,
                                  ap=sg[:, i0:i0 + 64].ap + [[0, 4]])
                              t_t = outp.tile([128, 64 * 4], dt.float32,
                                              tag="t")
                              t3 = t_t[:].rearrange("p (i w) -> p i w", w=4)
                              # t = sigma * P ; out = t + Q
                              nc.vector.tensor_tensor(
                                  out=t3[:, :, :], in0=pview[:, :, 0:4],
                                  in1=sg4, op=mybir.AluOpType.mult)
                              nc.vector.tensor_tensor(
                                  out=o3[:, :, :], in0=t3[:, :, :],
                                  in1=pview[:, :, 4:8],
                                  op=mybir.AluOpType.add)
                              # dma out: partition p, elements b*16384+p*128+(i0..i0+63)
                              nc.sync.dma_start(
                                  out=y[:, (b * EPP + i0) * 4:(b * EPP + i0 + 64) * 4],
                                  in_=t_o[:])
    nc.compile()
    return nc


_CACHE = {}


def kernel(x: np.ndarray, patterns: np.ndarray, results: np.ndarray) -> np.ndarray:
    import jax
    from jax.sharding import Mesh, PartitionSpec, NamedSharding
    from jax.experimental.shard_map import shard_map
    from concourse import mybir
    from concourse.bass2jax import (_bass_exec_p, install_neuronx_cc_hook,
                                    partition_id_tensor)

    x = np.asarray(x)
    patterns = np.asarray(patterns)
    results = np.asarray(results)
    rhs_luts = _build_luts(patterns, results)

    if "nc" not in _CACHE:
        _CACHE["nc"] = _build_kernel()
    nc = _CACHE["nc"]

    install_neuronx_cc_hook()
    partition_name = nc.partition_id_tensor.name if nc.partition_id_tensor else None
    in_names, out_names, out_avals, zero_outs = [], [], [], []
    for alloc in nc.m.functions[0].allocations:
        if not isinstance(alloc, mybir.MemoryLocationSet):
            continue
        name = alloc.memorylocations[0].name
        if alloc.kind == "ExternalInput":
            if name != partition_name:
                in_names.append(name)
        elif alloc.kind == "ExternalOutput":
            out_names.append(name)
            shape = tuple(alloc.tensor_shape)
            dtype = mybir.dt.np(alloc.dtype)
            out_avals.append(jax.core.ShapedArray(shape, dtype))
            zero_outs.append(np.zeros(shape, dtype))
    n_params = len(in_names)
    n_outs = len(out_avals)
    all_in_names = in_names + out_names + ([partition_name] if partition_name else [])

    def _body(*args):
        operands = list(args)
        if partition_name is not None:
            operands.append(partition_id_tensor())
        outs = _bass_exec_p.bind(
            *operands, out_avals=tuple(out_avals), in_names=tuple(all_in_names),
            out_names=tuple(out_names), lowering_input_output_aliases=(),
            sim_require_finite=False, sim_require_nnan=False, nc=nc)
        return tuple(outs)

    devices = jax.devices()[:N_CORES]
    mesh = Mesh(np.asarray(devices), ("core",))
    shard = NamedSharding(mesh, PartitionSpec("core"))
    fn = jax.jit(
        shard_map(_body, mesh=mesh,
                  in_specs=(PartitionSpec("core"),) * (n_params + n_outs),
                  out_specs=(PartitionSpec("core"),) * n_outs,
                  check_rep=False),
        keep_unused=True)

    # Build per-core input planes. Element n_loc = b*16384 + p*128 + i maps to
    # global n = core*N_LOC + n_loc; x dram row p = concat over b of 128 rows.
    xc = x.reshape(N_CORES, N_BLKS, 128, EPP * W_IN)          # [c, b, p, 128*8]
    x_in = np.ascontiguousarray(xc.transpose(0, 2, 1, 3)).reshape(
        N_CORES * 128, N_BLKS * EPP * W_IN)
    luts_in = np.broadcast_to(rhs_luts, (N_CORES, 128, 8)).reshape(
        N_CORES * 128, 8)
    arrays = {"x": x_in.astype(np.int32), "luts": np.ascontiguousarray(luts_in)}
    args = [jax.device_put(arrays[nm], shard) for nm in in_names]
    args += [jax.device_put(
        np.zeros((N_CORES * z.shape[0], *z.shape[1:]), z.dtype), shard)
        for z in zero_outs]
    out_arrs = fn(*args)
    yi = out_names.index("y")
    yv = np.asarray(out_arrs[yi]).reshape(N_CORES, 128, N_BLKS, EPP * W_OUT)
    # invert layout: [c, p, b, 128*4] -> [c, b, p, i, 4] -> n
    y_full = yv.transpose(0, 2, 1, 3).reshape(N, W_OUT)
    return y_full.astype(np.int32)



# revision 14
# speedup vs baseline: 10229.0610x; 1.0196x over previous
"""CBIndirectionLookup Trainium2 kernel.

Problem: x [N=2097152, 8] int32 bit-vectors; patterns [256, 8] (unique bit rows);
results [256, 4] int32. Output: results[argmax(all(x==patterns))] -> [N, 4] int32.

Strategy (pure data-parallel over 8 cores, N/8 = 262144 elems each):
 - Host: fold patterns+results into two 128-entry fp16 lut halves addressed by a
   7-bit code c7 (Horner over bits 0..6) plus sign sigma = 1-2*b7:
       out = Q[c7] + sigma * P[c7],  P = (lo-hi)/2, Q = (lo+hi)/2
 - Device per core, blocks of 16384 elements (x tile [128, 1024] i32):
   1. DVE tensor_tensor_scan (Horner) -> codes c7 (slot 6 of 8) and b7 (slot 7).
   2. PE transpose of strided c7 view -> code rows; GPSIMD partition_broadcast
      materializes each code row across 128 partitions (fp16).
   3. DVE tensor_scalar is_equal(bcast_codes, iota) -> one-hot weights W (fp16).
   4. PE: ldweights W[128,128] + matmul rhs=[P|Q] [128,8] -> psum [128 elems, 8].
   5. DVE recombine: out = Q + sigma*P (sigma natural layout) -> int32 sbuf.
   6. DMA out.
"""
import sys
sys.path.insert(0, "/opt/trn_rl_repo")

import numpy as np

N = 2_097_152
W_IN = 8
W_OUT = 4
P_TAB = 256
N_CORES = 8
N_LOC = N // N_CORES            # 262144 elements per core
BLK_ELEMS = 16384               # elements per block: [128, 128] layout
N_BLKS = N_LOC // BLK_ELEMS     # 16
EPP = BLK_ELEMS // 128          # 128 elements per partition per block
GROUP_BLKS = 4                  # blocks per code-transpose group (psum bank = 512)
N_GROUPS = N_BLKS // GROUP_BLKS


def _build_luts(patterns: np.ndarray, results: np.ndarray):
    """Host-side: fold the tiny tables into P/Q fp16 lut halves keyed by c7."""
    pat2idx = {}
    for p in range(patterns.shape[0]):
        pat2idx[tuple(int(v) for v in patterns[p])] = p
    lo = np.zeros((128, W_OUT), np.float64)
    hi = np.zeros((128, W_OUT), np.float64)
    for q in range(128):
        bits = [(q >> j) & 1 for j in range(7)]  # c7 = sum_j b_j 2^j
        p0 = pat2idx[tuple(bits + [0])]
        p1 = pat2idx[tuple(bits + [1])]
        lo[q] = results[p0]
        hi[q] = results[p1]
    Pm = (lo - hi) / 2.0
    Qm = (lo + hi) / 2.0
    rhs = np.concatenate([Pm, Qm], axis=1).astype(np.float16)  # [128, 8]
    assert np.array_equal(rhs.astype(np.float64), np.concatenate([Pm, Qm], 1))
    return rhs


def _build_kernel(repeats=1):
    import concourse.bass as bass
    import concourse.bacc as bacc
    import concourse.tile as tile
    from concourse import mybir

    nc = bacc.Bacc("TRN2", target_bir_lowering=False, debug=False,
                   num_devices=N_CORES)
    dt = mybir.dt
    x = nc.dram_tensor("x", [128, N_LOC // 128 * W_IN], dt.int32,
                       kind="ExternalInput").ap()
    luts = nc.dram_tensor("luts", [128, 8], dt.float16,
                          kind="ExternalInput").ap()
    y = nc.dram_tensor("y", [128, N_LOC // 128 * W_OUT], dt.int32,
                       kind="ExternalOutput").ap()
    ct_scratch = nc.dram_tensor("ct_scratch", [2, 128, GROUP_BLKS * 128],
                                dt.float16).ap()
    # x layout: core-row-major; partition p of block b holds elements
    #   n_loc = b*16384 + p*128 + i  (i in [0,128))
    # x dram row p = [block0: 128*8 | block1: 128*8 | ...] int32
    # y dram row p likewise with stride 4.

    with tile.TileContext(nc) as tc:
        with (
            tc.tile_pool(name="const", bufs=1) as constp,
            tc.tile_pool(name="xin", bufs=3) as xinp,
            tc.tile_pool(name="scan", bufs=3) as scanp,
            tc.tile_pool(name="sig", bufs=8) as sigp,
            tc.tile_pool(name="ct", bufs=2) as ctp,
            tc.tile_pool(name="ctr", bufs=2) as ctrp,
            tc.tile_pool(name="bc", bufs=6) as bcp,
            tc.tile_pool(name="w", bufs=6) as wp,
            tc.tile_pool(name="out", bufs=3) as outp,
            tc.tile_pool(name="pct", bufs=1, space="PSUM") as pctp,
            tc.tile_pool(name="pobc", bufs=3, space="PSUM") as pobc,
            tc.tile_pool(name="pg", bufs=1, space="PSUM") as pgp,
        ):
            t_luts = constp.tile([128, 8], dt.float16)
            nc.sync.dma_start(out=t_luts[:], in_=luts[:])
            t_iota32 = constp.tile([128, 1], dt.int32)
            nc.gpsimd.iota(t_iota32[:], pattern=[[0, 1]], base=0,
                           channel_multiplier=1)
            t_iota = constp.tile([128, 1], dt.float32)
            nc.vector.tensor_copy(t_iota[:], t_iota32[:])
            t_ident = constp.tile([128, 128], dt.float32)
            from concourse.masks import make_identity
            make_identity(nc, t_ident[:])
            # bit-weight pattern [1,2,4,...,64,0] per 8-group (c7 = sum b_j 2^j)
            t_pat = constp.tile([128, EPP * 8], dt.int32)
            for j in range(8):
                nc.vector.memset(
                    t_pat[:].rearrange("p (e k) -> p e k", k=8)[:, :, j:j + 1],
                    float(2 ** j) if j < 7 else 0.0)
            t_ones = constp.tile([1, 128], dt.float16)
            nc.vector.memset(t_ones[:], 1.0)

            import contextlib
            rep_cm = (tc.For_i(0, repeats, 1) if repeats > 1
                      else contextlib.nullcontext())
            with rep_cm:
              for g in range(N_GROUPS):
                  # --- per group: 4 blocks -> wide transposed code rows ---
                  t_ct_ps = pctp.tile([128, GROUP_BLKS * 128], dt.float32)
                  sigs = []
                  for bb in range(GROUP_BLKS):
                      b = g * GROUP_BLKS + bb
                      t_x = xinp.tile([128, EPP * 8], dt.int32)
                      nc.sync.dma_start(
                          out=t_x[:],
                          in_=x[:, b * EPP * 8:(b + 1) * EPP * 8])
                      x3 = t_x[:].rearrange("p (e k) -> p e k", k=8)
                      t_xw = scanp.tile([128, EPP * 8], dt.float32)
                      nc.vector.tensor_tensor(
                          out=t_xw[:], in0=t_x[:], in1=t_pat[:],
                          op=mybir.AluOpType.mult)
                      t_c7 = scanp.tile([128, EPP], dt.float32, tag="c7")
                      nc.vector.tensor_reduce(
                          t_c7[:], t_xw[:].rearrange("p (e k) -> p e k", k=8),
                          axis=mybir.AxisListType.X, op=mybir.AluOpType.add)
                      # sigma = 1 - 2*b7 (fp32) [128, EPP]
                      t_sg = sigp.tile([128, EPP], dt.float32)
                      nc.vector.tensor_scalar(
                          out=t_sg[:], in0=x3[:, :, 7],
                          scalar1=-2.0, scalar2=1.0,
                          op0=mybir.AluOpType.mult, op1=mybir.AluOpType.add)
                      sigs.append(t_sg)
                      # transpose c7 into group psum columns
                      nc.tensor.transpose(
                          t_ct_ps[:, bb * 128:(bb + 1) * 128],
                          t_c7[:], t_ident[:])
                  # escape transposed codes to sbuf fp16 rows
                  t_ct = ctp.tile([128, GROUP_BLKS * 128], dt.float16)
                  nc.vector.tensor_copy(t_ct[:], t_ct_ps[:])
                  nc.sync.dma_start(out=ct_scratch[g % 2], in_=t_ct[:])

                  # --- per code-row i: bcast, one-hot, 4 gather matmuls ---
                  t_pg = []
                  for par in range(2):
                      t_pg_buf = pgp.tile([128, 1024], dt.float32, tag=f"pg{par}")
                      t_pg.append(t_pg_buf)
                  ROWS_PER_Q = 32
                  for i in range(128):
                      if i % ROWS_PER_Q == 0:
                          t_ctrow = ctrp.tile([1, ROWS_PER_Q * GROUP_BLKS * 128],
                                              dt.float16, tag="ctrow")
                          nc.sync.dma_start(
                              out=t_ctrow[:],
                              in_=ct_scratch[g % 2, i:i + ROWS_PER_Q, :].rearrange(
                                  "r f -> (r f)")[None, :])
                      W_ROW = GROUP_BLKS * 128
                      row = t_ctrow[0:1,
                                    (i % ROWS_PER_Q) * W_ROW:(i % ROWS_PER_Q + 1) * W_ROW]
                      t_w = wp.tile([128, GROUP_BLKS * 128], dt.float16)
                      if i % 16 < 9:
                          t_bc = bcp.tile([128, GROUP_BLKS * 128], dt.float16)
                          nc.gpsimd.partition_broadcast(t_bc[:], row)
                          nc.vector.tensor_scalar(
                              out=t_w[:], in0=t_bc[:], scalar1=t_iota[:],
                              scalar2=None, op0=mybir.AluOpType.is_equal)
                      else:
                          t_obc = pobc.tile([128, GROUP_BLKS * 128], dt.float32)
                          nc.tensor.matmul(t_obc[:], t_ones[:], row,
                                           start=True, stop=True)
                          nc.vector.tensor_scalar(
                              out=t_w[:], in0=t_obc[:], scalar1=t_iota[:],
                              scalar2=None, op0=mybir.AluOpType.is_equal)
                      for bb in range(GROUP_BLKS):
                          ps = t_pg[bb % 2]
                          half = (bb // 2) * 512
                          nc.tensor.matmul(
                              ps[:, half + (i % 64) * 8: half + (i % 64) * 8 + 8],
                              t_w[:, bb * 128:(bb + 1) * 128],
                              t_luts[:],
                              start=True, stop=True)
                      # drain psum every 64 rows: recombine + dma out
                      if i % 64 == 63:
                          i0 = i - 63
                          for bb in range(GROUP_BLKS):
                              b = g * GROUP_BLKS + bb
                              ps = t_pg[bb % 2]
                              half = (bb // 2) * 512
                              pview = ps[:, half:half + 512].rearrange(
                                  "p (i w) -> p i w", w=8)
                              t_o = outp.tile([128, 64 * 4], dt.int32,
                                              tag="o")
                              o3 = t_o[:].rearrange("p (i w) -> p i w", w=4)
                              sg = sigs[bb]
                              sg4 = bass.AP(
                                  tensor=sg.tensor, offset=sg[:, i0:i0 + 64].offset,
                                  ap=sg[:, i0:i0 + 64].ap + [[0, 4]])
                              t_t = outp.tile([128, 64 * 4], dt.float32,
                                              tag="t")
                              t3 = t_t[:].rearrange("p (i w) -> p i w", w=4)
                              # t = sigma * P ; out = t + Q
                              nc.vector.tensor_tensor(
                                  out=t3[:, :, :], in0=pview[:, :, 0:4],
                                  in1=sg4, op=mybir.AluOpType.mult)
                              nc.vector.tensor_tensor(
                                  out=o3[:, :, :], in0=t3[:, :, :],
                                  in1=pview[:, :, 4:8],
                                  op=mybir.AluOpType.add)
                              # dma out: partition p, elements b*16384+p*128+(i0..i0+63)
                              nc.sync.dma_start(
                                  out=y[:, (b * EPP + i0) * 4:(b * EPP + i0 + 64) * 4],
                                  in_=t_o[:])
    nc.compile()
    return nc


_CACHE = {}


def kernel(x: np.ndarray, patterns: np.ndarray, results: np.ndarray) -> np.ndarray:
    import jax
    from jax.sharding import Mesh, PartitionSpec, NamedSharding
    from jax.experimental.shard_map import shard_map
    from concourse import mybir
    from concourse.bass2jax import (_bass_exec_p, install_neuronx_cc_hook,
                                    partition_id_tensor)

    x = np.asarray(x)
    patterns = np.asarray(patterns)
    results = np.asarray(results)
    rhs_luts = _build_luts(patterns, results)

    if "nc" not in _CACHE:
        _CACHE["nc"] = _build_kernel()
    nc = _CACHE["nc"]

    install_neuronx_cc_hook()
    partition_name = nc.partition_id_tensor.name if nc.partition_id_tensor else None
    in_names, out_names, out_avals, zero_outs = [], [], [], []
    for alloc in nc.m.functions[0].allocations:
        if not isinstance(alloc, mybir.MemoryLocationSet):
            continue
        name = alloc.memorylocations[0].name
        if alloc.kind == "ExternalInput":
            if name != partition_name:
                in_names.append(name)
        elif alloc.kind == "ExternalOutput":
            out_names.append(name)
            shape = tuple(alloc.tensor_shape)
            dtype = mybir.dt.np(alloc.dtype)
            out_avals.append(jax.core.ShapedArray(shape, dtype))
            zero_outs.append(np.zeros(shape, dtype))
    n_params = len(in_names)
    n_outs = len(out_avals)
    all_in_names = in_names + out_names + ([partition_name] if partition_name else [])

    def _body(*args):
        operands = list(args)
        if partition_name is not None:
            operands.append(partition_id_tensor())
        outs = _bass_exec_p.bind(
            *operands, out_avals=tuple(out_avals), in_names=tuple(all_in_names),
            out_names=tuple(out_names), lowering_input_output_aliases=(),
            sim_require_finite=False, sim_require_nnan=False, nc=nc)
        return tuple(outs)

    devices = jax.devices()[:N_CORES]
    mesh = Mesh(np.asarray(devices), ("core",))
    shard = NamedSharding(mesh, PartitionSpec("core"))
    fn = jax.jit(
        shard_map(_body, mesh=mesh,
                  in_specs=(PartitionSpec("core"),) * (n_params + n_outs),
                  out_specs=(PartitionSpec("core"),) * n_outs,
                  check_rep=False),
        keep_unused=True)

    # Build per-core input planes. Element n_loc = b*16384 + p*128 + i maps to
    # global n = core*N_LOC + n_loc; x dram row p = concat over b of 128 rows.
    xc = x.reshape(N_CORES, N_BLKS, 128, EPP * W_IN)          # [c, b, p, 128*8]
    x_in = np.ascontiguousarray(xc.transpose(0, 2, 1, 3)).reshape(
        N_CORES * 128, N_BLKS * EPP * W_IN)
    luts_in = np.broadcast_to(rhs_luts, (N_CORES, 128, 8)).reshape(
        N_CORES * 128, 8)
    arrays = {"x": x_in.astype(np.int32), "luts": np.ascontiguousarray(luts_in)}
    args = [jax.device_put(arrays[nm], shard) for nm in in_names]
    args += [jax.device_put(
        np.zeros((N_CORES * z.shape[0], *z.shape[1:]), z.dtype), shard)
        for z in zero_outs]
    out_arrs = fn(*args)
    yi = out_names.index("y")
    yv = np.asarray(out_arrs[yi]).reshape(N_CORES, 128, N_BLKS, EPP * W_OUT)
    # invert layout: [c, p, b, 128*4] -> [c, b, p, i, 4] -> n
    y_full = yv.transpose(0, 2, 1, 3).reshape(N, W_OUT)
    return y_full.astype(np.int32)

